# revision 7
# baseline (speedup 1.0000x reference)
"""Trainium2 Bass kernel for nn_CrossAttention1d (B=8, C=768, N=256, H=12, D=64).

Math (per batch b), algebraically equal to the reference but avoiding the
[3072, 3072] attention matrix via associativity:

    cp_full = W_proj @ cross_b + b_proj              [C, N]
    CP = cp_full.reshape(D, H*N)   (pure reshape)
    Xc = cross_b.reshape(D, H*N)   (pure reshape)
    K  = CP @ Xc^T                                   [D, D]
    X  = x_ori_b.reshape(D, H*N)
    OT = scale * K^T @ X                             [D, H*N]   (= O^T)
    out2T[h*64+d, n] = OT[d, n*12+h]                 [C, N]
    yT = W_dep @ out2T + b_dep                       [C, N]
    out_b = x_ori_b + yT

Sharding: data-parallel over batch, one batch per NeuronCore (8 cores).

On-chip schedule (per core):
  - proj computed transposed: cpT[n, o] = sum_c cross[c, n] wpT[c, o] (+ bias
    via a rank-1 K=1 matmul) so the K-matmul lhsT is a stride-12 free-dim
    slice of cpT (no transpose needed for CP).
  - crossT via 12 PE transposes (K-matmul rhs, also stride-12 slices).
  - K accumulated over 24 [128]x[64]x[64] matmuls; the attention scale is
    folded in during the PSUM->SBUF eviction, which also duplicates K to
    partitions [64:128] so OT matmuls can run on either partition half.
  - x loaded as [128, 1536] (p = half*64 + d, f = m - half*1536) for full
    DMA port width.
  - OT produced parity-split: OT2[d, t] = OT[d, 2t], OT2[64+d, t] = OT[d, 2t+1]
    by running each OT matmul twice with even/odd strided rhs, writing PSUM
    partitions [0:64] and [64:128].  The deproj rhs for c'-chunk j is then the
    single AP OT2[:, j::6] at full K=128.
  - deproj + b_dep rank-1 + residual add, store.

dtype variants: fp32 everywhere, or bf16 for the big DMA streams (weights,
cross, and the on-chip OT2) with fp32 PSUM accumulation throughout.
"""

import numpy as np

import concourse.bacc as bacc
import concourse.mybir as mybir
import concourse.tile as tile
from concourse.bass_utils import run_bass_kernel_spmd
from concourse.masks import make_identity

B, C, N = 8, 768, 256
H, D = 12, 64
M = H * N  # 3072
SCALE = float(D) ** -0.5
N_CORES = 8
F32 = mybir.dt.float32
BF16 = mybir.dt.bfloat16

USE_BF16 = True

_built_nc = None


def emit(tc, nc, xo, cr, wpT, wdT, bp, bd, out, bf16):
    """Emit one batch's worth of IR. DRAM handle args."""
    add = mybir.AluOpType.add
    Copy = mybir.ActivationFunctionType.Copy
    WDT = BF16 if bf16 else F32  # weight / cross / ot2 storage dtype

    with tc.tile_pool(name="sb", bufs=1) as sb:
        # ---- constants -------------------------------------------------
        ident = sb.tile([128, 128], WDT)
        make_identity(nc, ident[:])
        ones = sb.tile([1, 256], WDT)
        nc.gpsimd.memset(ones[:], 1.0)

        # ---- input DMAs ------------------------------------------------
        # cross in [128, (t n)] layout: cross_sb[p, t*256+n] = cr[t*128+p, n]
        cross_sb = sb.tile([128, 6 * N], WDT)
        nc.sync.dma_start(
            cross_sb[:].rearrange("p (t n) -> p t n", t=6),
            cr.ap().rearrange("(t p) n -> p t n", p=128),
        )

        # weights in [128, (t o)] layout, 2 big DMAs each
        wp_sb = sb.tile([128, 6 * C], WDT)
        wpT_r = wpT.ap().rearrange("(g t p) o -> p g t o", p=128, g=2)
        wp_v = wp_sb[:].rearrange("p (g t o) -> p g t o", g=2, t=3)
        wd_sb = sb.tile([128, 6 * C], WDT)
        wdT_r = wdT.ap().rearrange("(g t p) o -> p g t o", p=128, g=2)
        wd_v = wd_sb[:].rearrange("p (g t o) -> p g t o", g=2, t=3)
        for g in range(2):
            nc.sync.dma_start(wp_v[:, g], wpT_r[:, g])
        for g in range(2):
            nc.sync.dma_start(wd_v[:, g], wdT_r[:, g])

        # x in [128, 1536]: partition p = half*64 + d, col f = m - half*1536
        x_sb = sb.tile([128, M // 2], F32)
        xo_half = xo.ap().rearrange("(d half g) n -> half d (g n)", half=2, g=6)
        nc.sync.dma_start(x_sb[0:64, :], xo_half[0])
        nc.sync.dma_start(x_sb[64:128, :], xo_half[1])

        # biases as [1, C] rows
        bp_sb = sb.tile([1, C], WDT)
        nc.gpsimd.dma_start(bp_sb[:], bp.ap())
        bd_sb = sb.tile([1, C], WDT)
        nc.gpsimd.dma_start(bd_sb[:], bd.ap())

        # x again in residual layout [128, (t n)]
        xr_sb = sb.tile([128, 6 * N], F32)
        nc.sync.dma_start(
            xr_sb[:].rearrange("p (t n) -> p t n", t=6),
            xo.ap().rearrange("(t p) n -> p t n", p=128),
        )

        # ---- working SBUF tiles ---------------------------------------
        cpT_sb = sb.tile([128, 2 * C], F32)   # [n-chunk p, ni*768 + o]
        crT_sb = sb.tile([128, 2 * C], F32)   # [n-chunk p, ni*768 + c]
        k_sb = sb.tile([128, 64], F32)        # scale * K, duplicated halves
        ot2 = sb.tile([128, M // 2], WDT)     # parity-split OT
        out_sb = sb.tile([128, 6 * N], F32)

        # ---- proj (transposed) + crossT --------------------------------
        with (
            tc.tile_pool(name="ppj", bufs=4, space="PSUM") as ppj,
            tc.tile_pool(name="ptr", bufs=2, space="PSUM") as ptr,
        ):
            for ni in range(2):
                for oj in range(2):
                    ps = ppj.tile([128, 384], F32)
                    for t in range(6):
                        nc.tensor.matmul(
                            ps[:],
                            cross_sb[:, t * N + ni * 128: t * N + ni * 128 + 128],
                            wp_sb[:, t * C + oj * 384: t * C + oj * 384 + 384],
                            start=(t == 0),
                            stop=False,
                        )
                    # bias: cpT[n, o] += 1 * b_proj[o]
                    nc.tensor.matmul(
                        ps[:],
                        ones[0:1, 0:128],
                        bp_sb[0:1, oj * 384:(oj + 1) * 384],
                        start=False,
                        stop=True,
                    )
                    nc.vector.tensor_copy(
                        cpT_sb[:, ni * C + oj * 384: ni * C + oj * 384 + 384], ps[:]
                    )

            # crossT: 12 PE transposes of [128, 128] blocks
            for t in range(6):
                for ni in range(2):
                    pt = ptr.tile([128, 128], WDT)
                    nc.tensor.transpose(
                        pt[:],
                        cross_sb[:, t * N + ni * 128: t * N + ni * 128 + 128],
                        ident[:],
                    )
                    nc.scalar.activation(
                        crT_sb[:, ni * C + t * 128: ni * C + t * 128 + 128],
                        pt[:],
                        Copy,
                    )

        # ---- K / OT / deproj -------------------------------------------
        with (
            tc.tile_pool(name="pk", bufs=1, space="PSUM") as pk,
            tc.tile_pool(name="pot", bufs=3, space="PSUM") as pot,
            tc.tile_pool(name="py", bufs=2, space="PSUM") as py,
        ):
            # K[d', d] accumulated over (h, ni)
            kps = pk.tile([64, 64], F32)
            cpT_v = cpT_sb[:].rearrange("p (c d h) -> p c h d", c=2, h=H)
            crT_v = crT_sb[:].rearrange("p (c d h) -> p c h d", c=2, h=H)
            first = True
            for h in range(H):
                for ni in range(2):
                    nc.tensor.matmul(
                        kps[:],
                        cpT_v[:, ni, h],
                        crT_v[:, ni, h],
                        start=first,
                        stop=(h == H - 1 and ni == 1),
                    )
                    first = False
            # fold the attention scale in; duplicate K onto both halves
            nc.scalar.activation(k_sb[0:64, :], kps[:], Copy, scale=SCALE)
            nc.scalar.activation(k_sb[64:128, :], kps[:], Copy, scale=SCALE)

            # OT parity-split: even m -> partitions [0:64], odd m -> [64:128]
            x_v = x_sb[:].rearrange("p (t par) -> p par t", par=2)  # f = 2t+par
            for j in range(6):
                half, sub = j // 3, j % 3
                hb = half * 64
                po = pot.tile([128, 256], F32)
                nc.tensor.matmul(
                    po[0:64, :],
                    k_sb[hb:hb + 64, :],
                    x_v[hb:hb + 64, 0, sub * 256:(sub + 1) * 256],
                    start=True, stop=True,
                )
                nc.tensor.matmul(
                    po[64:128, :],
                    k_sb[hb:hb + 64, :],
                    x_v[hb:hb + 64, 1, sub * 256:(sub + 1) * 256],
                    start=True, stop=True,
                )
                nc.vector.tensor_copy(ot2[:, j * 256:(j + 1) * 256], po[:])

            # deproj + b_dep + residual
            ot2_v = ot2[:].rearrange("p (t six) -> p six t", six=6)
            for oi in range(6):
                yps = py.tile([128, 256], F32)
                for j in range(6):
                    nc.tensor.matmul(
                        yps[:],
                        wd_sb[:, j * C + oi * 128: j * C + oi * 128 + 128],
                        ot2_v[:, j],
                        start=(j == 0),
                        stop=False,
                    )
                nc.tensor.matmul(
                    yps[:],
                    bd_sb[0:1, oi * 128:(oi + 1) * 128],
                    ones[0:1, 0:256],
                    start=False,
                    stop=True,
                )
                nc.vector.tensor_tensor(
                    out_sb[:, oi * N:(oi + 1) * N],
                    yps[:],
                    xr_sb[:, oi * N:(oi + 1) * N],
                    add,
                )

        # ---- store -----------------------------------------------------
        out_r = out.ap().rearrange("(t p) n -> p t n", p=128)  # [128, 6, 256]
        out_v = out_sb[:].rearrange("p (t n) -> p t n", t=6)
        for s in range(3):
            nc.sync.dma_start(out_r[:, 2 * s:2 * s + 2], out_v[:, 2 * s:2 * s + 2])


def _declare(nc, bf16):
    WDT = BF16 if bf16 else F32
    xo = nc.dram_tensor("xo", [C, N], F32, kind="ExternalInput")
    cr = nc.dram_tensor("cr", [C, N], WDT, kind="ExternalInput")
    wpT = nc.dram_tensor("wpT", [C, C], WDT, kind="ExternalInput")
    wdT = nc.dram_tensor("wdT", [C, C], WDT, kind="ExternalInput")
    bp = nc.dram_tensor("bp", [1, C], WDT, kind="ExternalInput")
    bd = nc.dram_tensor("bd", [1, C], WDT, kind="ExternalInput")
    out = nc.dram_tensor("out", [C, N], F32, kind="ExternalOutput")
    return xo, cr, wpT, wdT, bp, bd, out


def build(bf16=USE_BF16):
    nc = bacc.Bacc("TRN2", target_bir_lowering=False, debug=False)
    args = _declare(nc, bf16)
    with tile.TileContext(nc) as tc:
        emit(tc, nc, *args, bf16)
    nc.compile()
    return nc


def build_loop(reps, bf16=USE_BF16):
    """Kernel body wrapped in a hardware For loop, for wall-clock timing."""
    nc = bacc.Bacc("TRN2", target_bir_lowering=False, debug=False)
    args = _declare(nc, bf16)
    with tile.TileContext(nc) as tc:
        with tc.For_i(0, reps, 1):
            emit(tc, nc, *args, bf16)
    nc.compile()
    return nc


def make_in_maps(x_ori, cross, W_proj, b_proj, W_dep, b_dep, bf16=USE_BF16):
    import ml_dtypes

    wdt = ml_dtypes.bfloat16 if bf16 else np.float32
    x_ori = np.ascontiguousarray(x_ori, np.float32)
    cross = np.ascontiguousarray(np.asarray(cross, np.float32), wdt)
    wpT = np.ascontiguousarray(np.asarray(W_proj, np.float32).T.astype(wdt))
    wdT = np.ascontiguousarray(np.asarray(W_dep, np.float32).T.astype(wdt))
    bp = np.ascontiguousarray(np.asarray(b_proj, np.float32).reshape(1, C), wdt)
    bd = np.ascontiguousarray(np.asarray(b_dep, np.float32).reshape(1, C), wdt)
    return [
        {
            "xo": x_ori[b],
            "cr": cross[b],
            "wpT": wpT,
            "wdT": wdT,
            "bp": bp,
            "bd": bd,
        }
        for b in range(B)
    ]


def kernel(**inputs):
    global _built_nc
    if _built_nc is None:
        _built_nc = build()
    nc = _built_nc
    in_maps = make_in_maps(
        inputs["x_ori"], inputs["cross"], inputs["W_proj"],
        inputs["b_proj"], inputs["W_dep"], inputs["b_dep"],
    )
    res = run_bass_kernel_spmd(nc, in_maps, list(range(N_CORES)))
    out = np.stack([np.asarray(res.results[c]["out"]) for c in range(N_CORES)])
    return out.astype(np.float32)


# revision 10
# speedup vs baseline: 1.1336x; 1.1336x over previous
"""Trainium2 Bass kernel for nn_CrossAttention1d (B=8, C=768, N=256, H=12, D=64).

Math (per batch b), algebraically equal to the reference but avoiding the
[3072, 3072] attention matrix via associativity:

    cp_full = W_proj @ cross_b + b_proj              [C, N]
    CP = cp_full.reshape(D, H*N)   (pure reshape)
    Xc = cross_b.reshape(D, H*N)   (pure reshape)
    K  = CP @ Xc^T                                   [D, D]
    X  = x_ori_b.reshape(D, H*N)
    OT = scale * K^T @ X                             [D, H*N]   (= O^T)
    out2T[h*64+d, n] = OT[d, n*12+h]                 [C, N]
    yT = W_dep @ out2T + b_dep                       [C, N]
    out_b = x_ori_b + yT

Sharding: data-parallel over batch, one batch per NeuronCore (8 cores).

On-chip schedule (per core):
  - proj computed transposed: cpT[n, o] = sum_c cross[c, n] wpT[c, o] (+ bias
    via a rank-1 K=1 matmul) so the K-matmul lhsT is a stride-12 free-dim
    slice of cpT (no transpose needed for CP).
  - crossT via 12 PE transposes (K-matmul rhs, also stride-12 slices).
  - K accumulated over 24 [128]x[64]x[64] matmuls; the attention scale is
    folded in during the PSUM->SBUF eviction, which also duplicates K to
    partitions [64:128] so OT matmuls can run on either partition half.
  - x loaded as [128, 1536] (p = half*64 + d, f = m - half*1536) for full
    DMA port width.
  - OT produced parity-split: OT2[d, t] = OT[d, 2t], OT2[64+d, t] = OT[d, 2t+1]
    by running each OT matmul twice with even/odd strided rhs, writing PSUM
    partitions [0:64] and [64:128].  The deproj rhs for c'-chunk j is then the
    single AP OT2[:, j::6] at full K=128.
  - deproj + b_dep rank-1 + residual add, store.

dtype variants: fp32 everywhere, or bf16 for the big DMA streams (weights,
cross, and the on-chip OT2) with fp32 PSUM accumulation throughout.
"""

import numpy as np

import concourse.bacc as bacc
import concourse.mybir as mybir
import concourse.tile as tile
from concourse.bass_utils import run_bass_kernel_spmd
from concourse.masks import make_identity

B, C, N = 8, 768, 256
H, D = 12, 64
M = H * N  # 3072
SCALE = float(D) ** -0.5
N_CORES = 8
F32 = mybir.dt.float32
BF16 = mybir.dt.bfloat16

USE_BF16 = True

_built_nc = None


def emit(tc, nc, xo, cr, wpT, wdT, bp, bd, out, bf16):
    """Emit one batch's worth of IR. DRAM handle args."""
    add = mybir.AluOpType.add
    Copy = mybir.ActivationFunctionType.Copy
    WDT = BF16 if bf16 else F32  # weight / cross / ot2 storage dtype

    with tc.tile_pool(name="sb", bufs=1) as sb:
        # ---- constants -------------------------------------------------
        ident = sb.tile([128, 128], WDT)
        make_identity(nc, ident[:])
        ones = sb.tile([1, 256], WDT)
        nc.gpsimd.memset(ones[:], 1.0)

        # ---- input DMAs ------------------------------------------------
        # cross in [128, (t n)] layout: cross_sb[p, t*256+n] = cr[t*128+p, n]
        cross_sb = sb.tile([128, 6 * N], WDT)
        nc.sync.dma_start(
            cross_sb[:].rearrange("p (t n) -> p t n", t=6),
            cr.ap().rearrange("(t p) n -> p t n", p=128),
        )

        # weights in [128, (t o)] layout, 2 big DMAs each
        wp_sb = sb.tile([128, 6 * C], WDT)
        wpT_r = wpT.ap().rearrange("(g t p) o -> p g t o", p=128, g=2)
        wp_v = wp_sb[:].rearrange("p (g t o) -> p g t o", g=2, t=3)
        wd_sb = sb.tile([128, 6 * C], WDT)
        wdT_r = wdT.ap().rearrange("(g t p) o -> p g t o", p=128, g=2)
        wd_v = wd_sb[:].rearrange("p (g t o) -> p g t o", g=2, t=3)
        for g in range(2):
            nc.sync.dma_start(wp_v[:, g], wpT_r[:, g])
        for g in range(2):
            nc.sync.dma_start(wd_v[:, g], wdT_r[:, g])

        # x in [128, 1536]: partition p = half*64 + d, col f = m - half*1536
        x_sb = sb.tile([128, M // 2], WDT)
        xo_half = xo.ap().rearrange("(d half g) n -> half d (g n)", half=2, g=6)
        nc.sync.dma_start(x_sb[0:64, :], xo_half[0])
        nc.sync.dma_start(x_sb[64:128, :], xo_half[1])

        # biases as [1, C] rows
        bp_sb = sb.tile([1, C], WDT)
        nc.gpsimd.dma_start(bp_sb[:], bp.ap())
        bd_sb = sb.tile([1, C], WDT)
        nc.gpsimd.dma_start(bd_sb[:], bd.ap())

        # x again in residual layout [128, (t n)]
        xr_sb = sb.tile([128, 6 * N], WDT)
        nc.sync.dma_start(
            xr_sb[:].rearrange("p (t n) -> p t n", t=6),
            xo.ap().rearrange("(t p) n -> p t n", p=128),
        )

        # ---- working SBUF tiles ---------------------------------------
        cpT_sb = sb.tile([128, 2 * C], F32)   # [n-chunk p, ni*768 + o]
        crT_sb = sb.tile([128, 2 * C], F32)   # [n-chunk p, ni*768 + c]
        k_sb = sb.tile([128, 64], WDT)        # scale * K, duplicated halves
        ot2 = sb.tile([128, M // 2], WDT)     # parity-split OT
        out_sb = sb.tile([128, 6 * N], WDT)

        # ---- proj (transposed) + crossT --------------------------------
        with (
            tc.tile_pool(name="ppj", bufs=4, space="PSUM") as ppj,
            tc.tile_pool(name="ptr", bufs=2, space="PSUM") as ptr,
        ):
            for ni in range(2):
                for oj in range(2):
                    ps = ppj.tile([128, 384], F32)
                    for t in range(6):
                        nc.tensor.matmul(
                            ps[:],
                            cross_sb[:, t * N + ni * 128: t * N + ni * 128 + 128],
                            wp_sb[:, t * C + oj * 384: t * C + oj * 384 + 384],
                            start=(t == 0),
                            stop=False,
                        )
                    # bias: cpT[n, o] += 1 * b_proj[o]
                    nc.tensor.matmul(
                        ps[:],
                        ones[0:1, 0:128],
                        bp_sb[0:1, oj * 384:(oj + 1) * 384],
                        start=False,
                        stop=True,
                    )
                    nc.vector.tensor_copy(
                        cpT_sb[:, ni * C + oj * 384: ni * C + oj * 384 + 384], ps[:]
                    )

            # crossT: 12 PE transposes of [128, 128] blocks
            for t in range(6):
                for ni in range(2):
                    pt = ptr.tile([128, 128], WDT)
                    nc.tensor.transpose(
                        pt[:],
                        cross_sb[:, t * N + ni * 128: t * N + ni * 128 + 128],
                        ident[:],
                    )
                    nc.scalar.activation(
                        crT_sb[:, ni * C + t * 128: ni * C + t * 128 + 128],
                        pt[:],
                        Copy,
                    )

        # ---- K / OT / deproj -------------------------------------------
        with (
            tc.tile_pool(name="pk", bufs=1, space="PSUM") as pk,
            tc.tile_pool(name="pot", bufs=3, space="PSUM") as pot,
            tc.tile_pool(name="py", bufs=2, space="PSUM") as py,
        ):
            # K[d', d] accumulated over (h, ni)
            kps = pk.tile([64, 64], F32)
            cpT_v = cpT_sb[:].rearrange("p (c d h) -> p c h d", c=2, h=H)
            crT_v = crT_sb[:].rearrange("p (c d h) -> p c h d", c=2, h=H)
            first = True
            for h in range(H):
                for ni in range(2):
                    nc.tensor.matmul(
                        kps[:],
                        cpT_v[:, ni, h],
                        crT_v[:, ni, h],
                        start=first,
                        stop=(h == H - 1 and ni == 1),
                    )
                    first = False
            # fold the attention scale in; duplicate K onto both halves
            nc.scalar.activation(k_sb[0:64, :], kps[:], Copy, scale=SCALE)
            nc.scalar.activation(k_sb[64:128, :], kps[:], Copy, scale=SCALE)

            # OT parity-split: even m -> partitions [0:64], odd m -> [64:128]
            x_v = x_sb[:].rearrange("p (t par) -> p par t", par=2)  # f = 2t+par
            for j in range(6):
                half, sub = j // 3, j % 3
                hb = half * 64
                po = pot.tile([128, 256], F32)
                nc.tensor.matmul(
                    po[0:64, :],
                    k_sb[hb:hb + 64, :],
                    x_v[hb:hb + 64, 0, sub * 256:(sub + 1) * 256],
                    start=True, stop=True,
                )
                nc.tensor.matmul(
                    po[64:128, :],
                    k_sb[hb:hb + 64, :],
                    x_v[hb:hb + 64, 1, sub * 256:(sub + 1) * 256],
                    start=True, stop=True,
                )
                nc.vector.tensor_copy(ot2[:, j * 256:(j + 1) * 256], po[:])

            # deproj + b_dep + residual
            ot2_v = ot2[:].rearrange("p (t six) -> p six t", six=6)
            for oi in range(6):
                yps = py.tile([128, 256], F32)
                for j in range(6):
                    nc.tensor.matmul(
                        yps[:],
                        wd_sb[:, j * C + oi * 128: j * C + oi * 128 + 128],
                        ot2_v[:, j],
                        start=(j == 0),
                        stop=False,
                    )
                nc.tensor.matmul(
                    yps[:],
                    bd_sb[0:1, oi * 128:(oi + 1) * 128],
                    ones[0:1, 0:256],
                    start=False,
                    stop=True,
                )
                nc.vector.tensor_tensor(
                    out_sb[:, oi * N:(oi + 1) * N],
                    yps[:],
                    xr_sb[:, oi * N:(oi + 1) * N],
                    add,
                )

        # ---- store -----------------------------------------------------
        out_r = out.ap().rearrange("(t p) n -> p t n", p=128)  # [128, 6, 256]
        out_v = out_sb[:].rearrange("p (t n) -> p t n", t=6)
        for s in range(3):
            nc.sync.dma_start(out_r[:, 2 * s:2 * s + 2], out_v[:, 2 * s:2 * s + 2])


def _declare(nc, bf16):
    WDT = BF16 if bf16 else F32
    xo = nc.dram_tensor("xo", [C, N], WDT, kind="ExternalInput")
    cr = nc.dram_tensor("cr", [C, N], WDT, kind="ExternalInput")
    wpT = nc.dram_tensor("wpT", [C, C], WDT, kind="ExternalInput")
    wdT = nc.dram_tensor("wdT", [C, C], WDT, kind="ExternalInput")
    bp = nc.dram_tensor("bp", [1, C], WDT, kind="ExternalInput")
    bd = nc.dram_tensor("bd", [1, C], WDT, kind="ExternalInput")
    out = nc.dram_tensor("out", [C, N], WDT, kind="ExternalOutput")
    return xo, cr, wpT, wdT, bp, bd, out


def build(bf16=USE_BF16):
    nc = bacc.Bacc("TRN2", target_bir_lowering=False, debug=False)
    args = _declare(nc, bf16)
    with tile.TileContext(nc) as tc:
        emit(tc, nc, *args, bf16)
    nc.compile()
    return nc


def build_loop(reps, bf16=USE_BF16):
    """Kernel body wrapped in a hardware For loop, for wall-clock timing."""
    nc = bacc.Bacc("TRN2", target_bir_lowering=False, debug=False)
    args = _declare(nc, bf16)
    with tile.TileContext(nc) as tc:
        with tc.For_i(0, reps, 1, hint_engines=(mybir.EngineType.PE,)):
            emit(tc, nc, *args, bf16)
    nc.compile()
    return nc


def make_in_maps(x_ori, cross, W_proj, b_proj, W_dep, b_dep, bf16=USE_BF16):
    import ml_dtypes

    wdt = ml_dtypes.bfloat16 if bf16 else np.float32
    x_ori_w = np.ascontiguousarray(np.asarray(x_ori, np.float32), wdt)
    cross = np.ascontiguousarray(np.asarray(cross, np.float32), wdt)
    wpT = np.ascontiguousarray(np.asarray(W_proj, np.float32).T.astype(wdt))
    wdT = np.ascontiguousarray(np.asarray(W_dep, np.float32).T.astype(wdt))
    bp = np.ascontiguousarray(np.asarray(b_proj, np.float32).reshape(1, C), wdt)
    bd = np.ascontiguousarray(np.asarray(b_dep, np.float32).reshape(1, C), wdt)
    return [
        {
            "xo": x_ori_w[b],
            "cr": cross[b],
            "wpT": wpT,
            "wdT": wdT,
            "bp": bp,
            "bd": bd,
        }
        for b in range(B)
    ]


def kernel(**inputs):
    global _built_nc
    if _built_nc is None:
        _built_nc = build()
    nc = _built_nc
    in_maps = make_in_maps(
        inputs["x_ori"], inputs["cross"], inputs["W_proj"],
        inputs["b_proj"], inputs["W_dep"], inputs["b_dep"],
    )
    res = run_bass_kernel_spmd(nc, in_maps, list(range(N_CORES)))
    out = np.stack([np.asarray(res.results[c]["out"]) for c in range(N_CORES)])
    return out.astype(np.float32)
